# revision 12
# baseline (speedup 1.0000x reference)
"""Bass/Trainium2 kernel for nn_MaskedLoss (MSE with bbox-ROI weighting).

Self-contained: hardcodes shapes (4,1,160,160,160) f32/i32, shards across
8 NeuronCores as (batch item, D-half) pairs, runs one SPMD Bass program
with one tiny pairwise AllReduce for the bbox exchange, and combines the
per-core partial sums on the host.

v3 (per core ~20.5 MB streamed; mask cast to bf16 on host — lossless for
the 0/1 mask):
  - DMA batched into 5-tile groups (5 entries/tensor, not 25) to amortize
    the ~630ns HWDGE entry overhead; mask goes first on both queues.
  - Mask column-any on PE (ones-matmul into one PSUM bank, 50-op
    accumulation group) reading a resident mega-mask tile.
  - d/h extrema with NO DRAM bounce: d(row), h(row) are static functions
    of (partition, tile, j), precomputed as coefficient tiles at setup;
    extrema = max-reduce of gt_rows * (BIG +- coord), one partition
    all-reduce for all four.
  - The collective's ~30us latency is hidden: all 5 cs (cumsum) group
    tiles stay resident, so only the 10 tiny extracts gate on the CC.
  - in_dh weights also from the coefficient tiles (no bounce).

The w-box is applied via cumulative-sum differences (cs[160v+RA] -
cs[160v+RB] per row, with a zero prefix column). Box bounds reproduce
the reference's float32 arithmetic exactly (k >= floor(x) <=> k > x-1,
plus a cast-mode-agnostic floor for the integer indices).
"""

import os
import sys

import numpy as np

sys.path.insert(0, "/opt/trn_rl_repo")

B, D, H, W = 4, 160, 160, 160
HALF_D = D // 2          # 80 d-slices per core
R = HALF_D * H           # 12800 rows (d,h) per core
KJ = 4                   # rows per partition line in a tile
NT = R // (128 * KJ)     # 25 tiles per tensor per core
GT = 5                   # tiles per DMA/compute group
NG = NT // GT            # 5 groups
GF = GT * KJ * W         # 3200 free elems per group
GV = GT * KJ             # 20 rows per partition line per group
N_CORES = 8
BIG = 1.0e6
W_OUT2 = 0.01            # W_OUT ** 2
EXPAND = 1.2

_CACHE: dict = {}


def _build_nc():
    from concourse import bacc, bass, bass_isa, tile
    import concourse.mybir as mybir

    f32 = mybir.dt.float32
    bf16 = mybir.dt.bfloat16
    i32 = mybir.dt.int32
    AX = mybir.AxisListType
    OP = mybir.AluOpType
    AF = mybir.ActivationFunctionType
    RO = bass_isa.ReduceOp

    nc = bacc.Bacc(
        "TRN2", target_bir_lowering=False, debug=False, num_devices=N_CORES
    )

    yp = nc.dram_tensor("yp", [R, W], f32, kind="ExternalInput")
    yt = nc.dram_tensor("yt", [R, W], f32, kind="ExternalInput")
    mk = nc.dram_tensor("mk", [R, W], bf16, kind="ExternalInput")
    meta = nc.dram_tensor("meta", [1], f32, kind="ExternalInput")
    out = nc.dram_tensor("out", [2], f32, kind="ExternalOutput")

    # grouped views: row r = 512*(5g+u) + 4p + j ; per (g,p) the free
    # layout is (u j w), contiguous 2560B runs in DRAM per (u,p)
    ypv = yp.ap().rearrange("(g u p j) w -> g p u j w", p=128, j=KJ, u=GT)
    ytv = yt.ap().rearrange("(g u p j) w -> g p u j w", p=128, j=KJ, u=GT)
    mkv = mk.ap().rearrange("(g u p j) w -> g p u j w", p=128, j=KJ, u=GT)

    with tile.TileContext(nc) as tc:
        with (
            tc.tile_pool(name="dram", bufs=1, space="DRAM") as dpool,
            tc.tile_pool(name="persist", bufs=1) as pp,
            tc.tile_pool(name="pp2", bufs=2) as ppool,
            tc.tile_pool(name="tp2", bufs=2) as tpool,
            tc.tile_pool(name="psp", bufs=1,
                         space=bass.MemorySpace.PSUM) as pspool,
            tc.tile_pool(name="sqp", bufs=2) as sqpool,
            tc.tile_pool(name="csp", bufs=NG) as cspool,
        ):
            cc1_in = dpool.tile([128], f32, tag="cc1_in")
            cc1_out = dpool.tile([128], f32, tag="cc1_out")

            from concourse.tile_rust import add_dep_helper

            # ---- setup: iotas/constants for the box math ----
            iota_w = pp.tile([1, W], i32, tag="iota_w")
            nc.gpsimd.iota(iota_w[:], pattern=[[1, W]], base=0,
                           channel_multiplier=0)
            k160 = pp.tile([1, W], f32, tag="k160")
            nc.vector.tensor_copy(out=k160[:], in_=iota_w[:])
            bmk = pp.tile([1, W], f32, tag="bmk")
            nc.vector.tensor_scalar(out=bmk[:], in0=k160[:], scalar1=-1.0,
                                    scalar2=BIG, op0=OP.mult, op1=OP.add)
            kpb = pp.tile([1, W], f32, tag="kpb")
            nc.vector.tensor_scalar(out=kpb[:], in0=k160[:], scalar1=BIG,
                                    scalar2=None, op0=OP.add)
            ones_bf = pp.tile([128, 1], bf16, tag="ones_bf")
            nc.gpsimd.memset(ones_bf[:], 1.0)

            meta_s = pp.tile([1, 1], f32, tag="meta_s")
            nc.gpsimd.dma_start(
                out=meta_s[:], in_=meta.ap().rearrange("(p x) -> p x", p=1))
            meta_b = pp.tile([128, 1], f32, tag="meta_b")
            nc.gpsimd.partition_broadcast(meta_b[:], meta_s[:], channels=128)

            # static coordinate tiles: r = 4p + 512t + j over (t,j) free dims
            iota_r = pp.tile([128, NT * KJ], i32, tag="iota_r")
            nc.gpsimd.iota(iota_r[:].rearrange("p (t j) -> p t j", j=KJ),
                           pattern=[[512, NT], [1, KJ]], base=0,
                           channel_multiplier=4)
            r_f = pp.tile([128, NT * KJ], f32, tag="r_f")
            nc.vector.tensor_copy(out=r_f[:], in_=iota_r[:])
            # d_loc = floor(r/160) via cast + correction (cast-mode agnostic)
            x160 = pp.tile([128, NT * KJ], f32, tag="x160")
            nc.vector.tensor_scalar(out=x160[:], in0=r_f[:],
                                    scalar1=1.0 / 160.0, scalar2=None,
                                    op0=OP.mult)
            d_i = pp.tile([128, NT * KJ], i32, tag="d_i")
            nc.vector.tensor_copy(out=d_i[:], in_=x160[:])
            d_f = pp.tile([128, NT * KJ], f32, tag="d_f")
            nc.vector.tensor_copy(out=d_f[:], in_=d_i[:])
            dcorr = pp.tile([128, NT * KJ], f32, tag="dcorr")
            nc.vector.tensor_tensor(out=dcorr[:], in0=d_f[:], in1=x160[:],
                                    op=OP.is_gt)
            d_loc = pp.tile([128, NT * KJ], f32, tag="d_loc")
            nc.vector.tensor_tensor(out=d_loc[:], in0=d_f[:], in1=dcorr[:],
                                    op=OP.subtract)
            h_t = pp.tile([128, NT * KJ], f32, tag="h_t")
            nc.vector.tensor_scalar(out=h_t[:], in0=d_loc[:], scalar1=-160.0,
                                    scalar2=None, op0=OP.mult)
            h_loc = pp.tile([128, NT * KJ], f32, tag="h_loc")
            nc.vector.tensor_tensor(out=h_loc[:], in0=r_f[:], in1=h_t[:],
                                    op=OP.add)
            d_gpc = pp.tile([128, NT * KJ], f32, tag="d_gpc")
            nc.vector.tensor_scalar(out=d_gpc[:], in0=d_loc[:],
                                    scalar1=meta_b[:, 0:1], scalar2=None,
                                    op0=OP.add)
            bmd_pc = pp.tile([128, NT * KJ], f32, tag="bmd_pc")
            nc.vector.tensor_scalar(out=bmd_pc[:], in0=d_gpc[:], scalar1=-1.0,
                                    scalar2=BIG, op0=OP.mult, op1=OP.add)
            dpb_pc = pp.tile([128, NT * KJ], f32, tag="dpb_pc")
            nc.vector.tensor_scalar(out=dpb_pc[:], in0=d_gpc[:], scalar1=BIG,
                                    scalar2=None, op0=OP.add)
            bmh_pc = pp.tile([128, NT * KJ], f32, tag="bmh_pc")
            nc.vector.tensor_scalar(out=bmh_pc[:], in0=h_loc[:], scalar1=-1.0,
                                    scalar2=BIG, op0=OP.mult, op1=OP.add)
            hpb_pc = pp.tile([128, NT * KJ], f32, tag="hpb_pc")
            nc.vector.tensor_scalar(out=hpb_pc[:], in0=h_loc[:], scalar1=BIG,
                                    scalar2=None, op0=OP.add)

            # ---------------- phase 1: mask projections ----------------
            # two resident mega-tiles (one per DMA queue) so the 5 slab
            # loads run on both queues in parallel
            mkA = pp.tile([128, 3 * GF], bf16, tag="mkA")  # slabs 0,2,4
            mkB = pp.tile([128, 2 * GF], bf16, tag="mkB")  # slabs 1,3
            acc_r = pp.tile([128, NT * KJ], bf16, tag="acc_r")
            colps = pspool.tile([1, 2 * W], f32, tag="colps")

            slab_ap = []
            mask_sync_last = None
            mask_scal_last = None
            for s in range(NG):
                if s % 2 == 0:
                    dst = mkA[:, (s // 2) * GF : (s // 2 + 1) * GF]
                    mask_sync_last = nc.sync.dma_start(
                        out=dst.rearrange("p (u j w) -> p u j w", u=GT, j=KJ),
                        in_=mkv[s])
                else:
                    dst = mkB[:, (s // 2) * GF : (s // 2 + 1) * GF]
                    mask_scal_last = nc.scalar.dma_start(
                        out=dst.rearrange("p (u j w) -> p u j w", u=GT, j=KJ),
                        in_=mkv[s])
                slab_ap.append(dst)

            with nc.allow_low_precision("0/1 mask row sums <=160 are exact"):
                for s in range(NG):
                    # row sums (DVE): [128, (v w)] -> [128, v]
                    nc.vector.tensor_reduce(
                        out=acc_r[:, s * GV : (s + 1) * GV],
                        in_=slab_ap[s].rearrange("p (v w) -> p v w", v=GV),
                        axis=AX.X, op=OP.add)
            # column sums (PE): 10 ones-matmuls per slab accumulated into
            # one PSUM bank; psum col n (n<320) collects flat (v w) cols
            # with v folded mod 2
            for s in range(NG):
                for c in range(GF // (2 * W)):
                    nc.tensor.matmul(
                        colps[:], ones_bf[:],
                        slab_ap[s][:, c * 2 * W : (c + 1) * 2 * W],
                        start=(s == 0 and c == 0),
                        stop=(s == NG - 1 and c == GF // (2 * W) - 1))

            # anyW from the PSUM accumulator: [1, (j2 w)] -> [1, W]
            anyw2 = pp.tile([1, 2 * W], f32, tag="anyw2")
            nc.scalar.activation(out=anyw2[:], in_=colps[:], func=AF.Copy)
            v_w = pp.tile([1, W], f32, tag="v_w")
            nc.gpsimd.tensor_tensor(out=v_w[:], in0=anyw2[:, 0:W],
                                    in1=anyw2[:, W : 2 * W], op=OP.add)

            # ---------------- CC prep (Pool only: mult/add/min/max) -----
            # indicators via min(count,1): counts are integer-valued >= 0
            gt_w = pp.tile([1, W], f32, tag="gt_w")
            nc.gpsimd.tensor_scalar(out=gt_w[:], in0=v_w[:], scalar1=1.0,
                                    scalar2=None, op0=OP.min)
            ta_w = pp.tile([1, W], f32, tag="ta_w")
            nc.gpsimd.tensor_tensor(out=ta_w[:], in0=gt_w[:], in1=bmk[:],
                                    op=OP.mult)
            ra_w = pp.tile([1, 1], f32, tag="ra_w")
            nc.gpsimd.tensor_reduce(out=ra_w[:], in_=ta_w[:],
                                    axis=AX.XYZWC, op=OP.max)
            tb_w = pp.tile([1, W], f32, tag="tb_w")
            nc.gpsimd.tensor_tensor(out=tb_w[:], in0=gt_w[:], in1=kpb[:],
                                    op=OP.mult)
            rb_w = pp.tile([1, 1], f32, tag="rb_w")
            nc.gpsimd.tensor_reduce(out=rb_w[:], in_=tb_w[:],
                                    axis=AX.XYZWC, op=OP.max)
            hf_sum = pp.tile([1, 1], f32, tag="hf_sum")
            nc.gpsimd.tensor_reduce(out=hf_sum[:], in_=gt_w[:],
                                    axis=AX.XYZWC, op=OP.add)
            hf_loc = pp.tile([1, 1], f32, tag="hf_loc")
            nc.gpsimd.tensor_scalar(out=hf_loc[:], in0=hf_sum[:], scalar1=1.0,
                                    scalar2=None, op0=OP.min)

            # d/h extrema straight from the static coordinate tiles
            gt_r = pp.tile([128, NT * KJ], f32, tag="gt_r")
            nc.gpsimd.tensor_scalar(out=gt_r[:], in0=acc_r[:], scalar1=1.0,
                                    scalar2=None, op0=OP.min)
            scr = pp.tile([128, NT * KJ], f32, tag="scr")
            e4 = []
            for k, coef in enumerate((bmd_pc, dpb_pc, bmh_pc, hpb_pc)):
                nc.gpsimd.tensor_tensor(out=scr[:], in0=gt_r[:],
                                        in1=coef[:], op=OP.mult)
                ek = pp.tile([1, 1], f32, tag=f"e4_{k}")
                nc.gpsimd.tensor_reduce(out=ek[:], in_=scr[:],
                                        axis=AX.XYZWC, op=OP.max)
                e4.append(ek)

            p8w = pp.tile([1, 128], f32, tag="p8w")
            nc.gpsimd.memset(p8w[:], 0.0)
            for slot, sl_src in ((0, ra_w[:]), (1, rb_w[:]), (2, hf_loc[:]),
                                 (3, e4[0][:]), (4, e4[1][:]),
                                 (5, e4[2][:]), (6, e4[3][:])):
                ofs = -BIG if slot != 2 else 0.0
                nc.gpsimd.tensor_scalar(out=p8w[:, slot : slot + 1],
                                        in0=sl_src, scalar1=ofs,
                                        scalar2=None, op0=OP.add)
            nc.gpsimd.dma_start(
                out=cc1_in[:].rearrange("(p x) -> p x", p=1), in_=p8w[:])
            nc.gpsimd.collective_compute(
                "AllReduce", OP.max,
                replica_groups=[[0, 1], [2, 3], [4, 5], [6, 7]],
                ins=[cc1_in[:].opt()], outs=[cc1_out[:].opt()])
            g8w = pp.tile([1, 128], f32, tag="g8w")
            nc.gpsimd.dma_start(
                out=g8w[:], in_=cc1_out[:].rearrange("(p x) -> p x", p=1))

            # w box -> integer cumsum-extract indices RA/RB.
            # slot algebra: g8w[0] = -mn_w, g8w[1] = mx_w
            # e = (mx - mn + 1)*0.6 ; c2 = mx + mn ; cpe = (c2 + 2e)/2 ...
            # computed with mult/add only (Pool-safe), floor on ACT.
            s01 = pp.tile([1, 1], f32, tag="s01")      # mx - mn
            nc.gpsimd.tensor_tensor(out=s01[:], in0=g8w[:, 0:1],
                                    in1=g8w[:, 1:2], op=OP.add)
            ew = pp.tile([1, 1], f32, tag="ew")        # (mx-mn+1)*0.5*1.2
            nc.gpsimd.tensor_scalar(out=ew[:], in0=s01[:], scalar1=1.0,
                                    scalar2=0.5, op0=OP.add, op1=OP.mult)
            nc.gpsimd.tensor_scalar(out=ew[:], in0=ew[:], scalar1=EXPAND,
                                    scalar2=None, op0=OP.mult)
            nmn = pp.tile([1, 1], f32, tag="nmn")      # mn_w
            nc.gpsimd.tensor_scalar(out=nmn[:], in0=g8w[:, 0:1], scalar1=-1.0,
                                    scalar2=None, op0=OP.mult)
            c2w = pp.tile([1, 1], f32, tag="c2w")      # mx + mn
            nc.gpsimd.tensor_tensor(out=c2w[:], in0=nmn[:], in1=g8w[:, 1:2],
                                    op=OP.add)
            cw = pp.tile([1, 1], f32, tag="cw")        # (mx+mn)/2
            nc.gpsimd.tensor_scalar(out=cw[:], in0=c2w[:], scalar1=0.5,
                                    scalar2=None, op0=OP.mult)
            new_ = pp.tile([1, 1], f32, tag="new_")    # -e
            nc.gpsimd.tensor_scalar(out=new_[:], in0=ew[:], scalar1=-1.0,
                                    scalar2=None, op0=OP.mult)
            cpe = pp.tile([1, 1], f32, tag="cpe")
            nc.gpsimd.tensor_tensor(out=cpe[:], in0=cw[:], in1=ew[:],
                                    op=OP.add)
            cme = pp.tile([1, 1], f32, tag="cme")
            nc.gpsimd.tensor_tensor(out=cme[:], in0=cw[:], in1=new_[:],
                                    op=OP.add)

            def floor_idx_act(x, tagp):
                # floor(x) as int32, entirely on ACT: cast, cast back,
                # corr = relu(sign(yf - x)), fl = yf - corr (exact ints)
                yi = pp.tile([1, 1], i32, tag=f"yi_{tagp}")
                nc.scalar.activation(out=yi[:], in_=x, func=AF.Copy)
                yf = pp.tile([1, 1], f32, tag=f"yf_{tagp}")
                nc.scalar.activation(out=yf[:], in_=yi[:], func=AF.Copy)
                nx = pp.tile([1, 1], f32, tag=f"nx_{tagp}")
                nc.scalar.activation(out=nx[:], in_=x, func=AF.Copy,
                                     scale=-1.0)
                sg = pp.tile([1, 1], f32, tag=f"sg_{tagp}")
                nc.scalar.activation(out=sg[:], in_=yf[:], func=AF.Sign,
                                     bias=nx[:])
                rc = pp.tile([1, 1], f32, tag=f"rc_{tagp}")
                nc.scalar.activation(out=rc[:], in_=sg[:], func=AF.Relu)
                fl = pp.tile([1, 1], f32, tag=f"fl_{tagp}")
                nc.scalar.activation(out=fl[:], in_=rc[:], func=AF.Identity,
                                     scale=-1.0, bias=yf[:])
                cl = pp.tile([1, 1], f32, tag=f"cl_{tagp}")
                nc.gpsimd.tensor_scalar(out=cl[:], in0=fl[:], scalar1=0.0,
                                        scalar2=float(W - 1), op0=OP.max,
                                        op1=OP.min)
                ii = pp.tile([1, 1], i32, tag=f"ii_{tagp}")
                nc.scalar.activation(out=ii[:], in_=cl[:], func=AF.Copy)
                return ii

            ra_i = floor_idx_act(cpe[:], "ra")
            rb_i = floor_idx_act(cme[:], "rb")
            reg_ra = nc.alloc_register(nc.scalar.engine, "reg_ra")
            nc.scalar.reg_load(reg_ra, ra_i[0:1, 0:1])
            rav = nc.scalar.snap(reg_ra, min_val=0, max_val=W - 1)
            reg_rb = nc.alloc_register(nc.scalar.engine, "reg_rb")
            nc.scalar.reg_load(reg_rb, rb_i[0:1, 0:1])
            rbv = nc.scalar.snap(reg_rb, min_val=0, max_val=W - 1)

            # ---------------- d/h bounds + in_dh weights ----------------
            def bounds(slot_mn, slot_mx, tagp):
                mn = pp.tile([1, 1], f32, tag=f"mn_{tagp}")
                nc.vector.tensor_scalar(out=mn[:],
                                        in0=g8w[:, slot_mn : slot_mn + 1],
                                        scalar1=-1.0, scalar2=None,
                                        op0=OP.mult)
                mx = g8w[:, slot_mx : slot_mx + 1]
                c2 = pp.tile([1, 1], f32, tag=f"c2_{tagp}")
                nc.vector.tensor_tensor(out=c2[:], in0=mn[:], in1=mx,
                                        op=OP.add)
                cC = pp.tile([1, 1], f32, tag=f"cC_{tagp}")
                nc.vector.tensor_scalar(out=cC[:], in0=c2[:], scalar1=0.5,
                                        scalar2=None, op0=OP.mult)
                em = pp.tile([1, 1], f32, tag=f"em_{tagp}")
                nc.vector.tensor_tensor(out=em[:], in0=mx, in1=mn[:],
                                        op=OP.subtract)
                nc.vector.tensor_scalar(out=em[:], in0=em[:], scalar1=1.0,
                                        scalar2=0.5, op0=OP.add, op1=OP.mult)
                eE = pp.tile([1, 1], f32, tag=f"eE_{tagp}")
                nc.vector.tensor_scalar(out=eE[:], in0=em[:], scalar1=EXPAND,
                                        scalar2=None, op0=OP.mult)
                lo = pp.tile([1, 1], f32, tag=f"lo_{tagp}")
                nc.vector.tensor_tensor(out=lo[:], in0=cC[:], in1=eE[:],
                                        op=OP.subtract)
                nc.vector.tensor_scalar(out=lo[:], in0=lo[:], scalar1=-1.0,
                                        scalar2=None, op0=OP.add)
                hi = pp.tile([1, 1], f32, tag=f"hi_{tagp}")
                nc.vector.tensor_tensor(out=hi[:], in0=cC[:], in1=eE[:],
                                        op=OP.add)
                nc.vector.tensor_scalar(out=hi[:], in0=hi[:], scalar1=-1.0,
                                        scalar2=float(W - 2), op0=OP.add,
                                        op1=OP.min)
                return lo, hi  # lo_m1, hi_m1 (compare form)

            lo_d, hi_d = bounds(3, 4, "d")
            lo_h, hi_h = bounds(5, 6, "h")

            # pack [lo_d, hi_d, lo_h, hi_h, hasfg] and broadcast to 128
            b5 = pp.tile([1, 5], f32, tag="b5")
            for k, src in enumerate((lo_d[:], hi_d[:], lo_h[:], hi_h[:],
                                     g8w[:, 2:3])):
                nc.vector.tensor_copy(out=b5[:, k : k + 1], in_=src)
            b5b = pp.tile([128, 5], f32, tag="b5b")
            nc.gpsimd.partition_broadcast(b5b[:], b5[:], channels=128)

            in_dh = pp.tile([128, NT * KJ], f32, tag="in_dh")
            wk1 = pp.tile([128, NT * KJ], f32, tag="wk1")
            nc.vector.tensor_scalar(out=in_dh[:], in0=d_gpc[:],
                                    scalar1=b5b[:, 0:1], scalar2=None,
                                    op0=OP.is_gt)
            nc.vector.tensor_scalar(out=wk1[:], in0=d_gpc[:],
                                    scalar1=b5b[:, 1:2], scalar2=None,
                                    op0=OP.is_le)
            nc.vector.tensor_tensor(out=in_dh[:], in0=in_dh[:], in1=wk1[:],
                                    op=OP.mult)
            nc.vector.tensor_scalar(out=wk1[:], in0=h_loc[:],
                                    scalar1=b5b[:, 2:3], scalar2=None,
                                    op0=OP.is_gt)
            nc.vector.tensor_tensor(out=in_dh[:], in0=in_dh[:], in1=wk1[:],
                                    op=OP.mult)
            nc.vector.tensor_scalar(out=wk1[:], in0=h_loc[:],
                                    scalar1=b5b[:, 3:4], scalar2=None,
                                    op0=OP.is_le)
            nc.vector.tensor_tensor(out=in_dh[:], in0=in_dh[:], in1=wk1[:],
                                    op=OP.mult)
            nc.vector.tensor_scalar(out=in_dh[:], in0=in_dh[:],
                                    scalar1=b5b[:, 4:5], scalar2=None,
                                    op0=OP.mult)  # fold has_fg

            # ---------------- phase 2: weighted MSE sums ----------------
            lp = nc.allow_low_precision("bf16 stream; fp32 accum")
            lp.__enter__()
            acc_tot = pp.tile([128, NG], f32, tag="acc_tot")
            acc_a = pp.tile([128, NT * KJ], f32, tag="acc_a")
            acc_b = pp.tile([128, NT * KJ], f32, tag="acc_b")
            for g in range(NG):
                p_g = ppool.tile([128, GF], f32, tag="p_g")
                yp_dma = nc.sync.dma_start(
                    out=p_g[:].rearrange("p (u j w) -> p u j w", u=GT, j=KJ),
                    in_=ypv[g])
                t_g = tpool.tile([128, GF], f32, tag="t_g")
                yt_dma = nc.scalar.dma_start(
                    out=t_g[:].rearrange("p (u j w) -> p u j w", u=GT, j=KJ),
                    in_=ytv[g])
                # bulk streams start only after the mask stream finishes
                add_dep_helper(yp_dma.ins, mask_sync_last.ins, sync=False,
                               reason="mask first on sync queue")
                add_dep_helper(yp_dma.ins, mask_scal_last.ins, sync=True,
                               reason="mask first (cross queue)")
                add_dep_helper(yt_dma.ins, mask_scal_last.ins, sync=False,
                               reason="mask first on scalar queue")
                add_dep_helper(yt_dma.ins, mask_sync_last.ins, sync=True,
                               reason="mask first (cross queue)")
                nc.vector.tensor_tensor(out=p_g[:], in0=p_g[:],
                                        in1=t_g[:], op=OP.subtract)
                sq_g = sqpool.tile([128, GF], f32, tag="sq_g")
                nc.scalar.activation(
                    out=sq_g[:], in_=p_g[:], func=AF.Square,
                    accum_out=acc_tot[:, g : g + 1])
                cs_g = cspool.tile([128, GF + 1], f32, tag="cs_g")
                nc.vector.memset(cs_g[:, 0:1], 0.0)
                scan_i = nc.vector.tensor_tensor_scan(
                    out=cs_g[:, 1 : GF + 1], data0=sq_g[:], data1=sq_g[:],
                    initial=0.0, op0=OP.add, op1=OP.bypass)
                nc.scalar.activation(
                    out=acc_a[:, g * GV : (g + 1) * GV],
                    in_=cs_g[:, bass.ds(rav, GV, W)], func=AF.Copy)
                nc.scalar.activation(
                    out=acc_b[:, g * GV : (g + 1) * GV],
                    in_=cs_g[:, bass.ds(rbv, GV, W)], func=AF.Copy)

            lp.__exit__(None, None, None)
            # ---------------- final reductions ----------------
            tot_col = pp.tile([128, 1], f32, tag="tot_col")
            nc.vector.tensor_reduce(out=tot_col[:], in_=acc_tot[:],
                                    axis=AX.X, op=OP.add)
            junk_a = pp.tile([128, NT * KJ], f32, tag="junk_a")
            sa_col = pp.tile([128, 1], f32, tag="sa_col")
            nc.vector.tensor_tensor(out=junk_a[:], in0=acc_a[:],
                                    in1=in_dh[:], op=OP.mult)
            nc.vector.tensor_reduce(out=sa_col[:], in_=junk_a[:], axis=AX.X,
                                    op=OP.add)
            junk_c = pp.tile([128, NT * KJ], f32, tag="junk_c")
            sb_col = pp.tile([128, 1], f32, tag="sb_col")
            nc.vector.tensor_tensor(out=junk_c[:], in0=acc_b[:],
                                    in1=in_dh[:], op=OP.mult)
            nc.vector.tensor_reduce(out=sb_col[:], in_=junk_c[:], axis=AX.X,
                                    op=OP.add)
            box_col = pp.tile([128, 1], f32, tag="box_col")
            nc.vector.tensor_tensor(out=box_col[:], in0=sa_col[:],
                                    in1=sb_col[:], op=OP.subtract)
            tot_r = pp.tile([128, 1], f32, tag="tot_r")
            nc.gpsimd.partition_all_reduce(tot_r[:], tot_col[:], channels=128,
                                           reduce_op=RO.add)
            box_r = pp.tile([128, 1], f32, tag="box_r")
            nc.gpsimd.partition_all_reduce(box_r[:], box_col[:], channels=128,
                                           reduce_op=RO.add)
            res2 = pp.tile([1, 2], f32, tag="res2")
            nc.vector.tensor_copy(out=res2[:, 0:1], in_=tot_r[0:1, :])
            nc.vector.tensor_copy(out=res2[:, 1:2], in_=box_r[0:1, :])
            nc.gpsimd.dma_start(
                out=out.ap().rearrange("(p x) -> p x", p=1), in_=res2[:])

    nc.compile()
    return nc


def get_nc():
    if "nc" not in _CACHE:
        _CACHE["nc"] = _build_nc()
    return _CACHE["nc"]


def make_in_maps(y_pred, y_true, mask):
    import ml_dtypes

    y_pred = np.asarray(y_pred, dtype=np.float32).reshape(B, D, H, W)
    y_true = np.asarray(y_true, dtype=np.float32).reshape(B, D, H, W)
    mask = np.asarray(mask, dtype=np.int32).reshape(B, D, H, W)
    mask_bf = mask.astype(ml_dtypes.bfloat16)  # 0/1 values: exact
    in_maps = []
    for c in range(N_CORES):
        b, half = c // 2, c % 2
        sl = slice(half * HALF_D, (half + 1) * HALF_D)
        in_maps.append({
            "yp": np.ascontiguousarray(y_pred[b, sl]).reshape(R, W),
            "yt": np.ascontiguousarray(y_true[b, sl]).reshape(R, W),
            "mk": np.ascontiguousarray(mask_bf[b, sl]).reshape(R, W),
            "meta": np.array([half * HALF_D], dtype=np.float32),
        })
    return in_maps


def combine(results):
    tot = 0.0
    box = 0.0
    for r in results:
        o = np.asarray(r["out"], dtype=np.float64).reshape(-1)
        tot += o[0]
        box += o[1]
    loss = (W_OUT2 * tot + (1.0 - W_OUT2) * box) / float(B * D * H * W)
    return np.array(loss, dtype=np.float32)


def kernel(y_pred, y_true, mask):
    from concourse.bass_utils import run_bass_kernel_spmd

    nc = get_nc()
    in_maps = make_in_maps(y_pred, y_true, mask)
    trace = bool(int(os.environ.get("BASS_KERNEL_TRACE", "0")))
    kwargs = {}
    if trace:
        kwargs = dict(trace=True, trace_cores=[0])
    res = run_bass_kernel_spmd(
        nc, in_maps, core_ids=list(range(N_CORES)), **kwargs
    )
    _CACHE["last_results"] = res
    return combine(res.results)
